# revision 30
# baseline (speedup 1.0000x reference)
"""CapsuleLayer (dynamic routing) Trainium2 Bass kernel.

x (128, 1152, 8) f32, W (1152, 32, 8, 16) f32 ->
  u_hat = einsum('bid,ijdk->bijk'); 3 routing iterations -> v (128, 32, 16).

Batch-sharded over 8 cores (16 b per core), W replicated, routing local.

Per core:
  phase 1: u_hat bf16 in SBUF, layout [p=(i%8)*16+b, f=(i//8, j, k)],
    via 144 matmuls with block-diagonal-x stationaries
    (K=(il,d)=64, M=(il,b)=128, N=(j,k)=512); iteration-0 weighted sum
    s0 = (1/32) sum_i u_hat via 144 dense accumulating matmuls (K=64,M=16).
  iterations 1,2:
    agreement b_ij = sum_k u*Vsum : DVE mul (bf16 2x) + bf16 fold tree.
    softmax over j: ACT exp (pair-packed, unnormalized) + DVE reduce/recip;
    1/z is folded into per-g PE stationaries (stats = ones_delta * rz)
    so no ACT broadcast-copy pass is needed.
    s = sum_i c*u : DVE mul e2 (pair-packed bf16) then PE accumulation with
    the rz-weighted delta stationaries (contracts partition dim (il,b) -> b).
    squash uses exp(0.5*ln(x)) for sqrt to stay in one ACT table.
"""

import numpy as np

B = 128
BL = 16  # batch per core
I = 1152
J = 32
D = 8
K = 16
JK = J * K  # 512
NCORES = 8
NG = I // 8      # 144 groups of 8 i's
GSUB = 24        # routing sub-blocks
GPS = NG // GSUB  # 6 groups per sub-block

_cached = {}
_last_in_maps = None


def _build_bass():
    import concourse.bass as bass
    import concourse.bacc as bacc_mod
    import concourse.tile as tile
    from concourse import mybir

    fp32 = mybir.dt.float32
    bf16 = mybir.dt.bfloat16
    AF = mybir.ActivationFunctionType
    ALU = mybir.AluOpType
    AX = mybir.AxisListType
    PF = mybir.PoolFunctionType

    nc = bacc_mod.Bacc()

    # s0 stream: [wt (512) | xs (16)] per pair; loaded first so v0 is
    # ready early and routing overlaps the u-matmul tail
    qs_d = nc.declare_dram_parameter(
        "qs", [NG // 2, 128, JK + BL], bf16, isOutput=False
    )
    # u stream: [wt (512) | xbd (128)] per pair
    q_d = nc.declare_dram_parameter("q", [NG // 2, 128, JK + 128], bf16, isOutput=False)
    ones_rep_d = nc.declare_dram_parameter("ones_rep", [BL, 128], bf16, isOutput=False)
    ones_sum_d = nc.declare_dram_parameter("ones_sum", [128, BL], bf16, isOutput=False)
    out_d = nc.declare_dram_parameter("out", [BL, J, K], fp32, isOutput=True)

    with tile.TileContext(nc) as tc:
        with (
            tc.tile_pool(name="big", bufs=1) as big,
            tc.tile_pool(name="consts", bufs=1) as consts,
            tc.tile_pool(name="wt", bufs=4) as wtp,
            tc.tile_pool(name="wts", bufs=2) as wtsp,
            tc.tile_pool(name="scr", bufs=2) as scr,
            tc.tile_pool(name="small", bufs=2) as small,
            tc.tile_pool(name="p2", bufs=3) as p2p,
            tc.tile_pool(name="pmul", bufs=2) as pmulp,
            tc.tile_pool(name="psum_u", bufs=5, space="PSUM") as psum_u_p,
            tc.tile_pool(name="psum_acc", bufs=1, space="PSUM") as psum_acc_p,
            tc.tile_pool(name="psum_misc", bufs=1, space="PSUM") as psum_misc_p,
        ):
            # ---------------- constants / staging ----------------
            ones_rep = consts.tile([BL, 128], bf16)   # delta[b, (il,b')]
            nc.gpsimd.dma_start(out=ones_rep, in_=ones_rep_d[:, :])
            ones_sum = consts.tile([128, BL], bf16)   # delta[(il,b), b']
            nc.gpsimd.dma_start(out=ones_sum, in_=ones_sum_d[:, :])

            U1 = big.tile([128, NG, J, K], bf16)

            psum_s0 = psum_acc_p.tile([BL, JK], fp32)

            # ---------------- phase 1a: s0 prefix ----------------
            # All 72 s0-matmuls run first from a lean (wt|xs) stream so
            # squash(s0) -> v0 -> vrep completes ~50us in; iteration 1's
            # DVE work then overlaps the remaining u-matmul/drain tail.
            QSW = JK + BL  # 528
            QSB = 2  # pairs per DMA batch
            NP = NG // 2  # 72 pairs
            for pb in range(NP // QSB):
                qs = wtsp.tile([128, QSB, QSW], bf16)
                nc.sync.dma_start(
                    out=qs,
                    in_=qs_d[pb * QSB : (pb + 1) * QSB].transpose([1, 0, 2]),
                )
                for h in range(QSB):
                    pp = pb * QSB + h
                    xs = qs[:, h, JK : JK + BL]
                    wtf = qs[:, h, 0:JK]
                    nc.tensor.matmul(
                        psum_s0, xs, wtf, start=(pp == 0), stop=(pp == NP - 1),
                        skip_group_check=True,
                    )

            # ---------------- phase 1b: u-matmul emitter ----------------
            # u-production is emitted incrementally: a prefix before routing
            # starts, the rest interleaved into iteration-1 rounds so the
            # PE/ACT tail hides under routing's DVE-bound rounds.
            QW = JK + 128  # 640
            QB = 2  # pairs per DMA batch (4 groups)
            emit_state = {"pb": 0}

            def emit_u_pairs(npb, split_drain=False):
                for _ in range(npb):
                    pb = emit_state["pb"]
                    if pb >= NP // QB:
                        return
                    emit_state["pb"] = pb + 1
                    qt = wtp.tile([128, QB, QW], bf16)
                    nc.sync.dma_start(
                        out=qt,
                        in_=q_d[pb * QB : (pb + 1) * QB].transpose([1, 0, 2]),
                    )
                    for h in range(QB):
                        pp = pb * QB + h
                        # two concurrent u-matmuls on row-group halves
                        for half in range(2):
                            g = 2 * pp + half
                            sl = slice(64 * half, 64 * half + 64)
                            wt = qt[sl, h, 0:JK]
                            xbd = qt[sl, h, JK : JK + 128]
                            pu = psum_u_p.tile([128, JK], fp32)
                            nc.tensor.matmul(
                                pu, xbd, wt, start=True, stop=True,
                                tile_position=(64 * half, 0),
                            )
                            dst = U1[:, g].rearrange("p j k -> p (j k)")
                            if split_drain:
                                # prefix drains split so the ACT queue isn't
                                # backlogged ahead of routing's first exps
                                nc.vector.tensor_copy(
                                    out=dst[:, 0:224], in_=pu[:, 0:224]
                                )
                                nc.scalar.copy(
                                    out=dst[:, 224:JK], in_=pu[:, 224:JK]
                                )
                            else:
                                # interleaved drains fully on ACT: keeps DVE
                                # free for the routing rounds this tail
                                # overlaps with (GPSIMD can't read PSUM)
                                nc.scalar.copy(out=dst[:, 0:JK], in_=pu[:, 0:JK])

            eps_tile = consts.tile([BL, 1], fp32)
            nc.vector.memset(eps_tile, 1e-9)

            # ---------------- squash helper ----------------
            def squash(psum_s, scale, vout_f32, psum_s_b=None):
                s_sb = small.tile([BL, J, K], fp32)
                nc.scalar.activation(
                    out=s_sb.rearrange("b j k -> b (j k)"),
                    in_=psum_s,
                    func=AF.Copy,
                    scale=float(scale),
                )
                if psum_s_b is not None:
                    nc.vector.scalar_tensor_tensor(
                        out=s_sb.rearrange("b j k -> b (j k)"),
                        in0=psum_s_b, scalar=float(scale),
                        in1=s_sb.rearrange("b j k -> b (j k)"),
                        op0=ALU.mult, op1=ALU.add,
                    )
                s2 = small.tile([BL, J, K], fp32)
                nc.vector.tensor_mul(s2, s_sb, s_sb)
                sq = small.tile([BL, J], fp32)
                nc.vector.tensor_reduce(out=sq, in_=s2, axis=AX.X, op=ALU.add)
                # sqrt(sq+eps) = exp(0.5*ln(sq+eps)): Ln/Exp/Copy share one
                # ACT table (natural_log_exp) so no ACT_TABLE_LOAD thrash
                lnt = small.tile([BL, J], fp32)
                nc.scalar.activation(out=lnt, in_=sq, func=AF.Ln, bias=eps_tile[:, :])
                rt = small.tile([BL, J], fp32)
                nc.scalar.activation(out=rt, in_=lnt, func=AF.Exp, scale=0.5)
                den = small.tile([BL, J], fp32)
                # den = (sq + 1) * rt in one fused op
                nc.vector.scalar_tensor_tensor(
                    out=den, in0=sq, scalar=1.0, in1=rt,
                    op0=ALU.add, op1=ALU.mult,
                )
                rec = small.tile([BL, J], fp32)
                nc.vector.reciprocal_approx_fast(out=rec, in_=den)
                fac = small.tile([BL, J], fp32)
                nc.vector.tensor_mul(fac, sq, rec)
                fac_b = fac[:, :].unsqueeze(2).to_broadcast([BL, J, K])
                nc.vector.tensor_tensor(out=vout_f32, in0=s_sb, in1=fac_b, op=ALU.mult)

            v_f32 = consts.tile([BL, J, K], fp32)
            Vsum = consts.tile([BL, J, K], fp32)
            squash(psum_s0, 1.0 / J, v_f32)
            nc.vector.tensor_copy(out=Vsum, in_=v_f32)

            vrep = consts.tile([128, J, K], bf16)  # Vsum replicated to (il,b)

            Vsum_bf = consts.tile([BL, JK], bf16)

            def build_vrep():
                nc.vector.tensor_copy(
                    out=Vsum_bf, in_=Vsum.rearrange("b j k -> b (j k)")
                )
                pv = psum_misc_p.tile([128, JK], fp32)
                nc.tensor.matmul(pv, ones_rep, Vsum_bf, start=True, stop=True)
                nc.scalar.copy(out=vrep.rearrange("p j k -> p (j k)"), in_=pv)

            build_vrep()

            # pre-produce a prefix of u groups AFTER build_vrep so vrep's
            # matmul isn't queued behind 52 u-matmuls on the PE; the rest
            # interleaves into iteration-1 rounds (52 + 4r >= 6r+6, all r)
            emit_u_pairs(26, split_drain=True)

            # ---------------- routing iterations ----------------
            # Software-pipelined: stages of sub s are emitted across
            # rounds so no engine queue ever blocks on a cross-engine dep.
            #   A(s):  DVE mul + f1 + f2, Pool f3
            #   B1(s): DVE f4 -> bij, ACT exp -> e
            #   B2(s): DVE Z-reduce + recip, ACT c2 = Copy(e * rz)
            #   C(s):  DVE p2 = c*u, PE s-matmuls
            for it in (1, 2):
                psum_s = psum_acc_p.tile([BL, JK], fp32)
                state = {}
                nmm = 0

                def stage_a(s):
                    g0 = s * GPS
                    prod = scr.tile([128, GPS, J, K], bf16)
                    vrep_b = vrep[:, :, :].unsqueeze(1).to_broadcast(
                        [128, GPS, J, K]
                    )
                    nc.vector.tensor_tensor(
                        out=prod, in0=U1[:, g0 : g0 + GPS], in1=vrep_b,
                        op=ALU.mult,
                    )
                    # fold tree stays on DVE: GPSIMD shares SBUF ports with
                    # DVE, so offloading big streams there slows both engines
                    nc.vector.tensor_tensor(
                        out=prod[:, :, :, 0:8], in0=prod[:, :, :, 0:8],
                        in1=prod[:, :, :, 8:16], op=ALU.add,
                    )
                    nc.vector.tensor_tensor(
                        out=prod[:, :, :, 0:4], in0=prod[:, :, :, 0:4],
                        in1=prod[:, :, :, 4:8], op=ALU.add,
                    )
                    state[s] = {"prod": prod}

                def stage_b1(s):
                    st = state[s]
                    prod = st["prod"]
                    bij = p2p.tile([128, GPS, J], fp32)
                    nc.vector.tensor_reduce(
                        out=bij, in_=prod[:, :, :, 0:4], axis=AX.X, op=ALU.add
                    )
                    # exp written pair-packed (unnormalized); the ACT
                    # accumulator produces z (x2, folded into ones_sum) so
                    # DVE does no z-reduce; 1/z rides the PE stationary
                    e2 = p2p.tile([128, GPS, J, 2], bf16)
                    z2 = p2p.tile([128, GPS], fp32)
                    for gg in range(GPS):
                        nc.scalar.activation(
                            out=e2[:, gg],
                            in_=bij[:, gg].unsqueeze(2).to_broadcast([128, J, 2]),
                            func=AF.Exp,
                            accum_out=z2[:, gg : gg + 1],
                        )
                    st["e2"] = e2
                    st["z2"] = z2

                def stage_b2(s):
                    st = state[s]
                    e2 = st["e2"]
                    rz = p2p.tile([128, GPS], fp32)
                    nc.vector.reciprocal_approx_fast(out=rz, in_=st["z2"])
                    # per-g softmax-normalizing stationaries:
                    # stats[p, g, b'] = ones_sum[p, b'] * rz[p, g]
                    stats = p2p.tile([128, GPS, BL], bf16)
                    nc.vector.tensor_tensor(
                        out=stats,
                        in0=ones_sum.unsqueeze(1).to_broadcast([128, GPS, BL]),
                        in1=rz.unsqueeze(2).to_broadcast([128, GPS, BL]),
                        op=ALU.mult,
                    )
                    st["stats"] = stats

                def stage_c(s):
                    nonlocal nmm
                    st = state.pop(s)
                    e2 = st["e2"]
                    stats = st["stats"]
                    g0 = s * GPS
                    p2 = pmulp.tile([128, GPS, J, K], bf16)
                    c_all = e2[:, :]  # [128, GPS, J, 2]
                    c_b = bass.AP(
                        tensor=c_all.tensor,
                        offset=c_all.offset,
                        ap=[c_all.ap[0], c_all.ap[1], c_all.ap[2],
                            [0, K // 2], [1, 2]],
                    )
                    nc.vector.tensor_tensor(
                        out=p2.rearrange(
                            "p g j (kk two) -> p g j kk two", two=2
                        ),
                        in0=U1[:, g0 : g0 + GPS].rearrange(
                            "p g j (kk two) -> p g j kk two", two=2
                        ),
                        in1=c_b,
                        op=ALU.mult,
                    )
                    for gg in range(GPS):
                        nc.tensor.matmul(
                            psum_s,
                            stats[:, gg],
                            p2[:, gg].rearrange("p j k -> p (j k)"),
                            start=(nmm == 0),
                            stop=(nmm == NG - 1),
                            skip_group_check=True,
                        )
                        nmm += 1

                for r in range(GSUB + 3):
                    if it == 1:
                        # trickle the remaining u-pairs under routing; 2
                        # pairs (4 groups) per round keeps production >=
                        # 32 + 4r groups vs consumption 6(r+1) groups
                        emit_u_pairs(2)
                    if r < GSUB:
                        stage_a(r)
                    if 1 <= r < GSUB + 1:
                        stage_b1(r - 1)
                    if 2 <= r < GSUB + 2:
                        stage_b2(r - 2)
                    if 3 <= r:
                        stage_c(r - 3)
                squash(psum_s, 1.0, v_f32)
                if it < 2:
                    nc.vector.tensor_add(Vsum, Vsum, v_f32)
                    build_vrep()

            nc.sync.dma_start(out=out_d[:, :, :], in_=v_f32)

    nc.finalize()
    return nc


def kernel(x: np.ndarray, W: np.ndarray) -> np.ndarray:
    from concourse.bass_utils import run_bass_kernel_spmd

    if "nc" not in _cached:
        _cached["nc"] = _build_bass()
    nc = _cached["nc"]

    x = np.ascontiguousarray(x, dtype=np.float32)
    W = np.ascontiguousarray(W, dtype=np.float32)
    # W as [g, (il d), (j k)]
    w_t = W.transpose(0, 2, 1, 3).reshape(NG, 64, JK)

    import ml_dtypes
    ones_rep_f = np.zeros((BL, 128), dtype=np.float32)
    for b in range(BL):
        for il in range(8):
            ones_rep_f[b, il * BL + b] = 1.0
    ones_rep = ones_rep_f.astype(ml_dtypes.bfloat16)
    # x2: the ACT-accumulated z sums both pair-packed exp copies (z2 = 2z),
    # so the softmax-normalizing stationaries carry 2/z2 = 1/z
    ones_sum = np.ascontiguousarray(2.0 * ones_rep_f.T).astype(ml_dtypes.bfloat16)

    in_maps = []
    for c in range(NCORES):
        xl = x[c * BL : (c + 1) * BL]  # [16, 1152, 8]
        blocks = xl.reshape(BL, NG, 8, D).transpose(1, 2, 3, 0)  # [g, il, d, b]
        xs_all = blocks.reshape(NG, 64, BL)
        xbd_all = np.zeros((NG, 8, D, 8, BL), dtype=np.float32)
        for il in range(8):
            xbd_all[:, il, :, il, :] = blocks[:, il]
        xbd_all = xbd_all.reshape(NG, 64, 128)
        import ml_dtypes
        q = np.concatenate([w_t, xbd_all], axis=2)
        q = np.ascontiguousarray(
            q.reshape(NG // 2, 2 * 64, JK + 128)
        ).astype(ml_dtypes.bfloat16)
        qs = np.concatenate([w_t, xs_all], axis=2)  # [NG, 64, 528]
        qs = np.ascontiguousarray(
            qs.reshape(NG // 2, 2 * 64, JK + BL)
        ).astype(ml_dtypes.bfloat16)
        in_maps.append(
            {
                "q": q,
                "qs": qs,
                "ones_rep": ones_rep,
                "ones_sum": ones_sum,
            }
        )
    global _last_in_maps
    _last_in_maps = in_maps
    res = run_bass_kernel_spmd(nc, in_maps, core_ids=list(range(NCORES)))
    outs = [res.results[c]["out"] for c in range(NCORES)]
    return np.concatenate(outs, axis=0).astype(np.float32)


if __name__ == "__main__":
    rng = np.random.default_rng(0)
    x = rng.standard_normal((B, I, D), dtype=np.float32)
    W = (rng.standard_normal((I, J, D, K)) * np.sqrt(2.0 / 24)).astype(np.float32)
    v = kernel(x, W)
    print(v.shape, v.dtype, float(np.abs(v).mean()))



# revision 32
# speedup vs baseline: 1.1539x; 1.1539x over previous
"""CapsuleLayer (dynamic routing) Trainium2 Bass kernel.

x (128, 1152, 8) f32, W (1152, 32, 8, 16) f32 ->
  u_hat = einsum('bid,ijdk->bijk'); 3 routing iterations -> v (128, 32, 16).

Batch-sharded over 8 cores (16 b per core), W replicated, routing local.

Per core:
  phase 1: u_hat bf16 in SBUF, layout [p=(i%8)*16+b, f=(i//8, j, k)],
    via 144 matmuls with block-diagonal-x stationaries
    (K=(il,d)=64, M=(il,b)=128, N=(j,k)=512); iteration-0 weighted sum
    s0 = (1/32) sum_i u_hat via 144 dense accumulating matmuls (K=64,M=16).
  iterations 1,2:
    agreement b_ij = sum_k u*Vsum : DVE mul (bf16 2x) + bf16 fold tree.
    softmax over j: ACT exp (pair-packed, unnormalized) + DVE reduce/recip;
    1/z is folded into per-g PE stationaries (stats = ones_delta * rz)
    so no ACT broadcast-copy pass is needed.
    s = sum_i c*u : DVE mul e2 (pair-packed bf16) then PE accumulation with
    the rz-weighted delta stationaries (contracts partition dim (il,b) -> b).
    squash uses exp(0.5*ln(x)) for sqrt to stay in one ACT table.
"""

import numpy as np

B = 128
BL = 16  # batch per core
I = 1152
J = 32
D = 8
K = 16
JK = J * K  # 512
NCORES = 8
NG = I // 8      # 144 groups of 8 i's
GSUB = 24        # routing sub-blocks
GPS = NG // GSUB  # 6 groups per sub-block

_cached = {}
_last_in_maps = None


def _build_bass():
    import concourse.bass as bass
    import concourse.bacc as bacc_mod
    import concourse.tile as tile
    from concourse import mybir

    fp32 = mybir.dt.float32
    bf16 = mybir.dt.bfloat16
    AF = mybir.ActivationFunctionType
    ALU = mybir.AluOpType
    AX = mybir.AxisListType
    PF = mybir.PoolFunctionType

    nc = bacc_mod.Bacc()

    # s0 stream: [wt (512) | xs (16)] per pair; loaded first so v0 is
    # ready early and routing overlaps the u-matmul tail
    qs_d = nc.declare_dram_parameter(
        "qs", [NG // 2, 128, JK + BL], bf16, isOutput=False
    )
    # u stream: [wt (512) | xbd (128)] per pair
    q_d = nc.declare_dram_parameter("q", [NG // 2, 128, JK + 128], bf16, isOutput=False)
    ones_rep_d = nc.declare_dram_parameter("ones_rep", [BL, 128], bf16, isOutput=False)
    ones_sum_d = nc.declare_dram_parameter("ones_sum", [128, BL], bf16, isOutput=False)
    out_d = nc.declare_dram_parameter("out", [BL, J, K], fp32, isOutput=True)

    with tile.TileContext(nc) as tc:
        with (
            tc.tile_pool(name="big", bufs=1) as big,
            tc.tile_pool(name="consts", bufs=1) as consts,
            tc.tile_pool(name="wt", bufs=4) as wtp,
            tc.tile_pool(name="wts", bufs=2) as wtsp,
            tc.tile_pool(name="scr", bufs=2) as scr,
            tc.tile_pool(name="small", bufs=2) as small,
            tc.tile_pool(name="p2", bufs=3) as p2p,
            tc.tile_pool(name="pmul", bufs=2) as pmulp,
            tc.tile_pool(name="psum_u", bufs=5, space="PSUM") as psum_u_p,
            tc.tile_pool(name="psum_acc", bufs=1, space="PSUM") as psum_acc_p,
            tc.tile_pool(name="psum_misc", bufs=1, space="PSUM") as psum_misc_p,
        ):
            # ---------------- constants / staging ----------------
            ones_rep = consts.tile([BL, 128], bf16)   # delta[b, (il,b')]
            nc.gpsimd.dma_start(out=ones_rep, in_=ones_rep_d[:, :])
            ones_sum = consts.tile([128, BL], bf16)   # delta[(il,b), b']
            nc.gpsimd.dma_start(out=ones_sum, in_=ones_sum_d[:, :])

            U1 = big.tile([128, NG, J, K], bf16)

            psum_s0 = psum_acc_p.tile([BL, JK], fp32)

            # ---------------- phase 1a: s0 prefix ----------------
            # All 72 s0-matmuls run first from a lean (wt|xs) stream so
            # squash(s0) -> v0 -> vrep completes ~50us in; iteration 1's
            # DVE work then overlaps the remaining u-matmul/drain tail.
            QSW = JK + BL  # 528
            QSB = 2  # pairs per DMA batch
            NP = NG // 2  # 72 pairs
            for pb in range(NP // QSB):
                qs = wtsp.tile([128, QSB, QSW], bf16)
                nc.sync.dma_start(
                    out=qs,
                    in_=qs_d[pb * QSB : (pb + 1) * QSB].transpose([1, 0, 2]),
                )
                for h in range(QSB):
                    pp = pb * QSB + h
                    xs = qs[:, h, JK : JK + BL]
                    wtf = qs[:, h, 0:JK]
                    nc.tensor.matmul(
                        psum_s0, xs, wtf, start=(pp == 0), stop=(pp == NP - 1),
                        skip_group_check=True,
                    )

            # ---------------- phase 1b: u-matmul emitter ----------------
            # u-production is emitted incrementally: a prefix before routing
            # starts, the rest interleaved into iteration-1 rounds so the
            # PE/ACT tail hides under routing's DVE-bound rounds.
            QW = JK + 128  # 640
            QB = 2  # pairs per DMA batch (4 groups)
            emit_state = {"pb": 0}

            def emit_u_pairs(npb, split_drain=False):
                for _ in range(npb):
                    pb = emit_state["pb"]
                    if pb >= NP // QB:
                        return
                    emit_state["pb"] = pb + 1
                    qt = wtp.tile([128, QB, QW], bf16)
                    nc.sync.dma_start(
                        out=qt,
                        in_=q_d[pb * QB : (pb + 1) * QB].transpose([1, 0, 2]),
                    )
                    for h in range(QB):
                        pp = pb * QB + h
                        # two concurrent u-matmuls on row-group halves
                        for half in range(2):
                            g = 2 * pp + half
                            sl = slice(64 * half, 64 * half + 64)
                            wt = qt[sl, h, 0:JK]
                            xbd = qt[sl, h, JK : JK + 128]
                            pu = psum_u_p.tile([128, JK], fp32)
                            nc.tensor.matmul(
                                pu, xbd, wt, start=True, stop=True,
                                tile_position=(64 * half, 0),
                            )
                            dst = U1[:, g].rearrange("p j k -> p (j k)")
                            if split_drain:
                                # prefix drains split so the ACT queue isn't
                                # backlogged ahead of routing's first exps
                                nc.vector.tensor_copy(
                                    out=dst[:, 0:224], in_=pu[:, 0:224]
                                )
                                nc.scalar.copy(
                                    out=dst[:, 224:JK], in_=pu[:, 224:JK]
                                )
                            else:
                                # interleaved drains fully on ACT: keeps DVE
                                # free for the routing rounds this tail
                                # overlaps with (GPSIMD can't read PSUM)
                                nc.scalar.copy(out=dst[:, 0:JK], in_=pu[:, 0:JK])

            eps_tile = consts.tile([BL, 1], fp32)
            nc.vector.memset(eps_tile, 1e-9)

            # ---------------- squash helper ----------------
            def squash(psum_s, scale, vout_f32, psum_s_b=None):
                s_sb = small.tile([BL, J, K], fp32)
                nc.scalar.activation(
                    out=s_sb.rearrange("b j k -> b (j k)"),
                    in_=psum_s,
                    func=AF.Copy,
                    scale=float(scale),
                )
                if psum_s_b is not None:
                    nc.vector.scalar_tensor_tensor(
                        out=s_sb.rearrange("b j k -> b (j k)"),
                        in0=psum_s_b, scalar=float(scale),
                        in1=s_sb.rearrange("b j k -> b (j k)"),
                        op0=ALU.mult, op1=ALU.add,
                    )
                s2 = small.tile([BL, J, K], fp32)
                nc.vector.tensor_mul(s2, s_sb, s_sb)
                sq = small.tile([BL, J], fp32)
                nc.vector.tensor_reduce(out=sq, in_=s2, axis=AX.X, op=ALU.add)
                # sqrt(sq+eps) = exp(0.5*ln(sq+eps)): Ln/Exp/Copy share one
                # ACT table (natural_log_exp) so no ACT_TABLE_LOAD thrash
                lnt = small.tile([BL, J], fp32)
                nc.scalar.activation(out=lnt, in_=sq, func=AF.Ln, bias=eps_tile[:, :])
                rt = small.tile([BL, J], fp32)
                nc.scalar.activation(out=rt, in_=lnt, func=AF.Exp, scale=0.5)
                den = small.tile([BL, J], fp32)
                # den = (sq + 1) * rt in one fused op
                nc.vector.scalar_tensor_tensor(
                    out=den, in0=sq, scalar=1.0, in1=rt,
                    op0=ALU.add, op1=ALU.mult,
                )
                rec = small.tile([BL, J], fp32)
                nc.vector.reciprocal_approx_fast(out=rec, in_=den)
                fac = small.tile([BL, J], fp32)
                nc.vector.tensor_mul(fac, sq, rec)
                fac_b = fac[:, :].unsqueeze(2).to_broadcast([BL, J, K])
                nc.vector.tensor_tensor(out=vout_f32, in0=s_sb, in1=fac_b, op=ALU.mult)

            v_f32 = consts.tile([BL, J, K], fp32)
            Vsum = consts.tile([BL, J, K], fp32)
            squash(psum_s0, 1.0 / J, v_f32)
            nc.vector.tensor_copy(out=Vsum, in_=v_f32)

            vrep = consts.tile([128, J, K], bf16)  # Vsum replicated to (il,b)

            Vsum_bf = consts.tile([BL, JK], bf16)

            def build_vrep():
                nc.vector.tensor_copy(
                    out=Vsum_bf, in_=Vsum.rearrange("b j k -> b (j k)")
                )
                pv = psum_misc_p.tile([128, JK], fp32)
                nc.tensor.matmul(pv, ones_rep, Vsum_bf, start=True, stop=True)
                nc.scalar.copy(out=vrep.rearrange("p j k -> p (j k)"), in_=pv)

            build_vrep()

            # pre-produce a prefix of u groups AFTER build_vrep so vrep's
            # matmul isn't queued behind 52 u-matmuls on the PE; the rest
            # interleaves into iteration-1 rounds (52 + 4r >= 6r+6, all r)
            emit_u_pairs(26, split_drain=True)

            # ---------------- routing iterations ----------------
            # Software-pipelined: stages of sub s are emitted across
            # rounds so no engine queue ever blocks on a cross-engine dep.
            #   A(s):  DVE mul + f1 + f2, Pool f3
            #   B1(s): DVE f4 -> bij, ACT exp -> e
            #   B2(s): DVE Z-reduce + recip, ACT c2 = Copy(e * rz)
            #   C(s):  DVE p2 = c*u, PE s-matmuls
            for it in (1, 2):
                psum_s = psum_acc_p.tile([BL, JK], fp32)
                state = {}
                nmm = 0

                def stage_a(s):
                    g0 = s * GPS
                    prod = scr.tile([128, GPS, J, K], bf16)
                    vrep_b = vrep[:, :, :].unsqueeze(1).to_broadcast(
                        [128, GPS, J, K]
                    )
                    nc.vector.tensor_tensor(
                        out=prod, in0=U1[:, g0 : g0 + GPS], in1=vrep_b,
                        op=ALU.mult,
                    )
                    # fold tree stays on DVE: GPSIMD shares SBUF ports with
                    # DVE, so offloading big streams there slows both engines
                    nc.vector.tensor_tensor(
                        out=prod[:, :, :, 0:8], in0=prod[:, :, :, 0:8],
                        in1=prod[:, :, :, 8:16], op=ALU.add,
                    )
                    nc.vector.tensor_tensor(
                        out=prod[:, :, :, 0:4], in0=prod[:, :, :, 0:4],
                        in1=prod[:, :, :, 4:8], op=ALU.add,
                    )
                    state[s] = {"prod": prod}

                def stage_b1(s):
                    st = state[s]
                    prod = st["prod"]
                    bij = p2p.tile([128, GPS, J], fp32)
                    nc.vector.tensor_reduce(
                        out=bij, in_=prod[:, :, :, 0:4], axis=AX.X, op=ALU.add
                    )
                    # exp written pair-packed (unnormalized); 1/z rides the
                    # PE stationary instead of an ACT broadcast-copy pass.
                    # (ACT accum_out was tried for z and costs 734ns/op on
                    # HW -- slower than the DVE z-reduce.)
                    e2 = p2p.tile([128, GPS, J, 2], bf16)
                    nc.scalar.activation(
                        out=e2,
                        in_=bij.unsqueeze(3).to_broadcast([128, GPS, J, 2]),
                        func=AF.Exp,
                    )
                    st["e2"] = e2

                def stage_b2(s):
                    st = state[s]
                    e2 = st["e2"]
                    z = p2p.tile([128, GPS], fp32)
                    nc.vector.tensor_reduce(
                        out=z, in_=e2[:, :, :, 0], axis=AX.X, op=ALU.add
                    )
                    rz = p2p.tile([128, GPS], fp32)
                    nc.vector.reciprocal_approx_fast(out=rz, in_=z)
                    # per-g softmax-normalizing stationaries:
                    # stats[p, g, b'] = ones_sum[p, b'] * rz[p, g]
                    stats = p2p.tile([128, GPS, BL], bf16)
                    nc.vector.tensor_tensor(
                        out=stats,
                        in0=ones_sum.unsqueeze(1).to_broadcast([128, GPS, BL]),
                        in1=rz.unsqueeze(2).to_broadcast([128, GPS, BL]),
                        op=ALU.mult,
                    )
                    st["stats"] = stats

                def stage_c(s):
                    nonlocal nmm
                    st = state.pop(s)
                    e2 = st["e2"]
                    stats = st["stats"]
                    g0 = s * GPS
                    p2 = pmulp.tile([128, GPS, J, K], bf16)
                    c_all = e2[:, :]  # [128, GPS, J, 2]
                    c_b = bass.AP(
                        tensor=c_all.tensor,
                        offset=c_all.offset,
                        ap=[c_all.ap[0], c_all.ap[1], c_all.ap[2],
                            [0, K // 2], [1, 2]],
                    )
                    nc.vector.tensor_tensor(
                        out=p2.rearrange(
                            "p g j (kk two) -> p g j kk two", two=2
                        ),
                        in0=U1[:, g0 : g0 + GPS].rearrange(
                            "p g j (kk two) -> p g j kk two", two=2
                        ),
                        in1=c_b,
                        op=ALU.mult,
                    )
                    for gg in range(GPS):
                        nc.tensor.matmul(
                            psum_s,
                            stats[:, gg],
                            p2[:, gg].rearrange("p j k -> p (j k)"),
                            start=(nmm == 0),
                            stop=(nmm == NG - 1),
                            skip_group_check=True,
                        )
                        nmm += 1

                for r in range(GSUB + 3):
                    if it == 1:
                        # trickle the remaining u-pairs under routing; 2
                        # pairs (4 groups) per round keeps production >=
                        # 32 + 4r groups vs consumption 6(r+1) groups
                        emit_u_pairs(2)
                    if r < GSUB:
                        stage_a(r)
                    if 1 <= r < GSUB + 1:
                        stage_b1(r - 1)
                    if 2 <= r < GSUB + 2:
                        stage_b2(r - 2)
                    if 3 <= r:
                        stage_c(r - 3)
                squash(psum_s, 1.0, v_f32)
                if it < 2:
                    nc.vector.tensor_add(Vsum, Vsum, v_f32)
                    build_vrep()

            nc.sync.dma_start(out=out_d[:, :, :], in_=v_f32)

    nc.finalize()
    return nc


def kernel(x: np.ndarray, W: np.ndarray) -> np.ndarray:
    from concourse.bass_utils import run_bass_kernel_spmd

    if "nc" not in _cached:
        _cached["nc"] = _build_bass()
    nc = _cached["nc"]

    x = np.ascontiguousarray(x, dtype=np.float32)
    W = np.ascontiguousarray(W, dtype=np.float32)
    # W as [g, (il d), (j k)]
    w_t = W.transpose(0, 2, 1, 3).reshape(NG, 64, JK)

    import ml_dtypes
    ones_rep_f = np.zeros((BL, 128), dtype=np.float32)
    for b in range(BL):
        for il in range(8):
            ones_rep_f[b, il * BL + b] = 1.0
    ones_rep = ones_rep_f.astype(ml_dtypes.bfloat16)
    ones_sum = np.ascontiguousarray(ones_rep_f.T).astype(ml_dtypes.bfloat16)

    in_maps = []
    for c in range(NCORES):
        xl = x[c * BL : (c + 1) * BL]  # [16, 1152, 8]
        blocks = xl.reshape(BL, NG, 8, D).transpose(1, 2, 3, 0)  # [g, il, d, b]
        xs_all = blocks.reshape(NG, 64, BL)
        xbd_all = np.zeros((NG, 8, D, 8, BL), dtype=np.float32)
        for il in range(8):
            xbd_all[:, il, :, il, :] = blocks[:, il]
        xbd_all = xbd_all.reshape(NG, 64, 128)
        import ml_dtypes
        q = np.concatenate([w_t, xbd_all], axis=2)
        q = np.ascontiguousarray(
            q.reshape(NG // 2, 2 * 64, JK + 128)
        ).astype(ml_dtypes.bfloat16)
        qs = np.concatenate([w_t, xs_all], axis=2)  # [NG, 64, 528]
        qs = np.ascontiguousarray(
            qs.reshape(NG // 2, 2 * 64, JK + BL)
        ).astype(ml_dtypes.bfloat16)
        in_maps.append(
            {
                "q": q,
                "qs": qs,
                "ones_rep": ones_rep,
                "ones_sum": ones_sum,
            }
        )
    global _last_in_maps
    _last_in_maps = in_maps
    res = run_bass_kernel_spmd(nc, in_maps, core_ids=list(range(NCORES)))
    outs = [res.results[c]["out"] for c in range(NCORES)]
    return np.concatenate(outs, axis=0).astype(np.float32)


if __name__ == "__main__":
    rng = np.random.default_rng(0)
    x = rng.standard_normal((B, I, D), dtype=np.float32)
    W = (rng.standard_normal((I, J, D, K)) * np.sqrt(2.0 / 24)).astype(np.float32)
    v = kernel(x, W)
    print(v.shape, v.dtype, float(np.abs(v).mean()))

